# revision 1
# baseline (speedup 1.0000x reference)
"""Trainium2 Bass kernel for nn_Attention (RMSNorm + QKV + 16-head attention + out-proj).

Sharding: 8 cores = 4 batches x 2 head-groups (DP x TP). Each core gets one
batch element and 8 of the 16 heads, computes a partial out-projection
([2048, 1024]); the host sums the two head-group partials per batch.

Per-core pipeline (T=2048 tokens, D=1024; all matmul operands bf16 with fp32
PSUM accumulation -- bf16 streams ~3x faster than fp32/f32r on this PE and
measured end-to-end error is ~6e-3 scale-relative):
  P1  load x, RMS-normalize (gamma*sqrt(D)*dh^-0.5 folded into the weights on
      the host), PE-transpose to feature-major xnT [128, 8fc, T].
  P2a v token-major [128, 16tt, (4pair, 2head, 65)]: matmul(lhsT=xnT-tile,
      rhs=Wv); a ones column is appended per head so the AV matmul (M=65)
      also produces the softmax denominator in row 64 for free.
  P2b qT/kT head-pair-major [128 (2x64 dims), T] via matmul(lhsT=W, rhs=xnT).
  P3  attention per (pair, 512-wide q chunk): S^T tiles [128 keys, 2x512]
      with the two heads row-packed on the PE (K=64 at array rows 0-63 /
      64-127); one ScalarE exp over both banks (the phase bottleneck,
      ~1.06us/slot); AV accumulates per head into separate banks (this
      walrus rejects matmul outputs at non-zero partitions, so no column
      packing); 1/denom is broadcast across partitions with a tiny f32r
      ones-matmul; normalization tails are emitted one group late so their
      PE work never head-of-line-blocks the S stream; head-1 results are
      DMA-shifted to partitions 64:127 to form the out-projection pair tile.
  P4  out-projection: matmul(lhsT=attn pair tile, rhs=w_out rows),
      accumulated over the 4 pairs in PSUM.

Toolchain workarounds: sync waits are capped at 1 per instruction (excess
moved onto NoOps via a BIR JSON post-pass) because this walrus rejects
multi-wait encodings; fp32r is used only where precision matters (1/denom
broadcast); gpsimd custom ops and DMA partition-broadcast are unavailable.
"""

import json
import numpy as np

B, T, D = 4, 2048, 1024
HEADS, DH = 16, 64
NT = T // 128   # 16 token tiles
FC = D // 128   # 8 feature chunks
NPAIR = 4       # head pairs per core (8 heads)
QCN = 4         # q chunks of 512
KT = NT         # key tiles

_PROG = {}

# ---------------------------------------------------------------------------
# BIR post-pass: this walrus build rejects >1 sync wait per instruction in
# some encodings; move excess waits onto NoOps inserted before the offender.
_MAX_WAITS = 2
# opcodes whose walrus encoding only fits one sync wait
_ONE_WAIT_OPS = ()


def _split_excess_waits(bir_json: bytes) -> bytes:
    d = json.loads(bir_json)
    changed = False
    for fn in d.get("functions", []):
        for blk in fn.get("blocks", []):
            new_insts = []
            for inst in blk.get("instructions", []):
                si = inst.get("sync_info") or {}
                waits = si.get("on_wait") or []
                _MAX_WAITS = 1
                if len(waits) > _MAX_WAITS:
                    changed = True
                    excess = waits[: len(waits) - _MAX_WAITS]
                    si["on_wait"] = waits[len(waits) - _MAX_WAITS:]
                    inst["sync_info"] = si
                    for k in range(0, len(excess), _MAX_WAITS):
                        new_insts.append({
                            "debug": inst.get("debug", 0),
                            "engine": inst["engine"],
                            "ins": [],
                            "name": f"{inst['name']}-wsplit{k}",
                            "opcode": "NoOp",
                            "outs": [],
                            "sync_info": {
                                "on_update": [],
                                "on_wait": excess[k : k + _MAX_WAITS],
                            },
                        })
                new_insts.append(inst)
            blk["instructions"] = new_insts
    if not changed:
        return bir_json
    return json.dumps(d).encode()


def _install_bir_patch():
    import concourse.bass2jax as bass2jax
    import concourse.bass_utils as bass_utils

    if getattr(bass2jax.compile_bir_kernel, "_is_waitsplit_patch", False):
        return
    orig = bass_utils.compile_bir_kernel

    def patched(bir_json, tmpdir, neff_name="file.neff"):
        return orig(_split_excess_waits(bir_json), tmpdir, neff_name)

    patched._is_waitsplit_patch = True
    bass2jax.compile_bir_kernel = patched
    bass_utils.compile_bir_kernel = patched


# ---------------------------------------------------------------------------


def _build_program(loop_n=1, phases="all"):
    from contextlib import ExitStack

    import concourse.bass as bass
    import concourse.mybir as mybir
    import concourse.tile as tile
    from concourse.masks import make_identity

    F32 = mybir.dt.float32
    F32R = mybir.dt.float32r
    BF16 = mybir.dt.bfloat16
    AF = mybir.ActivationFunctionType
    OP = mybir.AluOpType

    nc = bass.Bass("TRN2", target_bir_lowering=False, debug=False, num_devices=8)
    x_d = nc.dram_tensor("x", [T, D], F32, kind="ExternalInput").ap()
    wq_d = nc.dram_tensor("wq", [128, FC, 512], BF16, kind="ExternalInput").ap()
    wk_d = nc.dram_tensor("wk", [128, FC, 512], BF16, kind="ExternalInput").ap()
    wv_d = nc.dram_tensor("wv", [128, FC, 512], BF16, kind="ExternalInput").ap()
    wo_d = nc.dram_tensor("wo", [128, NPAIR, D], BF16, kind="ExternalInput").ap()
    out_d = nc.dram_tensor("out", [T, D], F32, kind="ExternalOutput").ap()

    with tile.TileContext(nc) as tc:
        with ExitStack() as es:
            singles = es.enter_context(tc.tile_pool(name="singles", bufs=1))
            qpool = es.enter_context(tc.tile_pool(name="qp", bufs=1))
            kpool = es.enter_context(tc.tile_pool(name="kp", bufs=1))
            vpool = es.enter_context(tc.tile_pool(name="vp", bufs=1))

            ident = singles.tile([128, 128], F32)
            make_identity(nc, ident[:])
            ident_bf = singles.tile([128, 128], BF16)
            nc.vector.tensor_copy(ident_bf[:], ident[:])
            ones_f32 = singles.tile([128, 64], F32)
            nc.gpsimd.memset(ones_f32[:], 1.0)
            ones_r = singles.tile([128, 64], F32R)
            nc.vector.tensor_copy(ones_r[:], ones_f32[:])
            stats = singles.tile([128, 64], F32)
            sqscratch = singles.tile([128, D], F32)

            # v with a ones column appended per head: AV matmul with M=65
            # yields attn_out rows 0:64 plus the softmax denominator in row 64
            v_sb = vpool.tile([128, NT, NPAIR, 2, 65], BF16)
            nc.vector.tensor_copy(
                v_sb[:, :, :, :, 64:65],
                ones_f32[:, 0:1].broadcast_to([128, NT, NPAIR, 2, 1]))
            aopool = es.enter_context(tc.tile_pool(name="aout", bufs=1))
            attn_sb = aopool.tile([128, NPAIR, T], BF16)

            import contextlib
            loop_ctx = (tc.For_i(0, loop_n, 1) if loop_n > 1
                        else contextlib.nullcontext())
            with loop_ctx:
                # PSUM pools for the projection phases
                psA_es = es.enter_context(ExitStack())
                ps_acc = psA_es.enter_context(
                    tc.tile_pool(name="ps_acc", bufs=2, space="PSUM"))

                # SBUF frame that is released before the attention phase
                xnt_es = es.enter_context(ExitStack())
                xnt_pool = xnt_es.enter_context(tc.tile_pool(name="xnt", bufs=1))
                xnT = xnt_pool.tile([128, FC, T], BF16)

                # ---- P1: load x, RMS-normalize (gamma*sqrt(D) folded into the
                # weights), transpose to feature-major.
                with tc.tile_pool(name="tp", bufs=4, space="PSUM") as tpool, \
                     tc.tile_pool(name="xin", bufs=3) as xp:
                    for tt in range(NT):
                        x_t = xp.tile([128, D], F32, tag="x")
                        nc.sync.dma_start(x_t[:], x_d[tt * 128:(tt + 1) * 128, :])
                        ss = stats[:, tt:tt + 1]
                        nc.scalar.activation(
                            sqscratch[:], x_t[:], AF.Square, accum_out=ss)
                        nrm = stats[:, 16 + tt:17 + tt]
                        nc.scalar.sqrt(nrm, ss)
                        nc.vector.tensor_scalar_max(nrm, nrm, 1e-12)
                        rinv = stats[:, 32 + tt:33 + tt]
                        nc.vector.reciprocal(rinv, nrm)
                        xn_b = xp.tile([128, D], BF16, tag="xb")
                        nc.vector.tensor_scalar_mul(xn_b[:], x_t[:], rinv)
                        for fc in range(FC):
                            pt = tpool.tile([128, 128], BF16, tag="t")
                            nc.tensor.transpose(
                                pt[:], xn_b[:, fc * 128:(fc + 1) * 128], ident_bf[:])
                            dst = xnT[:, fc, tt * 128:(tt + 1) * 128]
                            if fc % 2 == 0:
                                nc.vector.tensor_copy(dst, pt[:])
                            else:
                                nc.scalar.copy(dst, pt[:])

                # ---- P2a: v (token-major)
                with tc.tile_pool(name="wvp", bufs=1) as wvp:
                    wv_sb = wvp.tile([128, FC, 512], BF16)
                    nc.sync.dma_start(wv_sb[:], wv_d[:])
                    for tt in range(NT):
                        pv = ps_acc.tile([128, 512], F32, tag="acc")
                        for fc in range(FC):
                            nc.tensor.matmul(
                                pv[:], xnT[:, fc, tt * 128:(tt + 1) * 128],
                                wv_sb[:, fc, :],
                                start=(fc == 0), stop=(fc == FC - 1))
                        nc.vector.tensor_copy(
                        v_sb[:, tt, :, :, 0:64],
                        pv[:].rearrange("p (pr h c) -> p pr h c", pr=NPAIR, h=2))

                # ---- P2b: q/k projections (serial; acc pool then closes so
                # attention gets 8 PSUM banks: 4 S + 4 AV double-buffered)
                qTf = qpool.tile([128, NPAIR, T], BF16)
                kTf = kpool.tile([128, NPAIR, T], BF16)
                with tc.tile_pool(name="wqp", bufs=2) as wqp, \
                     tc.tile_pool(name="wkp", bufs=2) as wkp:
                    for p in range(NPAIR):
                        wq_p = wqp.tile([128, FC, 128], BF16, tag="wq")
                        nc.sync.dma_start(wq_p[:], wq_d[:, :, p * 128:(p + 1) * 128])
                        wk_p = wkp.tile([128, FC, 128], BF16, tag="wk")
                        nc.sync.dma_start(wk_p[:], wk_d[:, :, p * 128:(p + 1) * 128])
                        for qc in range(QCN):
                            cs = slice(qc * 512, (qc + 1) * 512)
                            pq = ps_acc.tile([128, 512], F32, tag="acc")
                            for fc in range(FC):
                                nc.tensor.matmul(
                                    pq[:], wq_p[:, fc, :], xnT[:, fc, cs],
                                    start=(fc == 0), stop=(fc == FC - 1))
                            nc.vector.tensor_copy(qTf[:, p, cs], pq[:])
                            pk = ps_acc.tile([128, 512], F32, tag="acc")
                            for fc in range(FC):
                                nc.tensor.matmul(
                                    pk[:], wk_p[:, fc, :], xnT[:, fc, cs],
                                    start=(fc == 0), stop=(fc == FC - 1))
                            nc.vector.tensor_copy(kTf[:, p, cs], pk[:])
                xnt_es.close()
                psA_es.close()

                # ---- P3: attention; AV double-buffered, normalization tails
                # lagged one group so their PE work never blocks the S stream
                att_es = es.enter_context(ExitStack())
                ps_s = att_es.enter_context(
                    tc.tile_pool(name="ps_s", bufs=2, space="PSUM"))
                ps_av = att_es.enter_context(
                    tc.tile_pool(name="ps_av", bufs=2, space="PSUM"))
                p_pool = att_es.enter_context(tc.tile_pool(name="pp", bufs=6))
                rcp_pool = att_es.enter_context(tc.tile_pool(name="rcp", bufs=2))
                bsb_pool = att_es.enter_context(tc.tile_pool(name="bsb", bufs=2))
                stg_pool = att_es.enter_context(tc.tile_pool(name="stg", bufs=2))

                def emit_rcp(pAV):
                    # reciprocal of the denominator row; issued as soon as the
                    # AV group stops so the lagged pB matmuls never wait on DVE
                    rcp = rcp_pool.tile([65, 1024], F32R, tag="rcp")
                    with nc.allow_low_precision(reason="1/denom feeds f32r matmul"):
                        nc.vector.reciprocal(rcp[64:65, :], pAV[64:65, :])
                    return rcp

                def emit_tail(p, qc, pAV, rcp):
                    cs = slice(qc * 512, (qc + 1) * 512)
                    pBa = ps_s.tile([128, 1024], F32, tag="s")
                    nc.tensor.matmul(
                        pBa[0:64, 0:512], ones_r[64:65, :], rcp[64:65, 0:512],
                        start=True, stop=True, tile_position=(64, 0))
                    nc.tensor.matmul(
                        pBa[0:64, 512:1024], ones_r[64:65, :], rcp[64:65, 512:1024],
                        start=True, stop=True, tile_position=(64, 0))
                    bsb = bsb_pool.tile([64, 1024], F32, tag="b")
                    nc.vector.tensor_copy(bsb[:], pBa[0:64, :])
                    nc.vector.tensor_tensor(
                        attn_sb[0:64, p, cs], pAV[0:64, 0:512], bsb[:, 0:512],
                        OP.mult)
                    stg = stg_pool.tile([64, 512], BF16, tag="stg")
                    nc.vector.tensor_tensor(
                        stg[:], pAV[0:64, 512:1024], bsb[:, 512:1024], OP.mult)
                    nc.sync.dma_start(attn_sb[64:128, p, cs], stg[:])

                pending = None
                for p in range(NPAIR):
                    qT = qTf[:, p, :]
                    kT = kTf[:, p, :]
                    for qc in range(QCN):
                        cs = slice(qc * 512, (qc + 1) * 512)
                        pAV = ps_av.tile([128, 1024], F32, tag="av")
                        for kt in range(KT):
                            ks = slice(kt * 128, (kt + 1) * 128)
                            pS = ps_s.tile([128, 1024], F32, tag="s")
                            nc.tensor.matmul(
                                pS[:, 0:512], kT[0:64, ks], qT[0:64, cs],
                                start=True, stop=True, tile_position=(0, 0))
                            nc.tensor.matmul(
                                pS[:, 512:1024], kT[64:128, ks], qT[64:128, cs],
                                start=True, stop=True, tile_position=(64, 0))
                            p_t = p_pool.tile([128, 1024], BF16, tag="P")
                            nc.scalar.activation(p_t[:], pS[:], AF.Exp)
                            # AV with ones-augmented v: rows 0:64 attn_out,
                            # row 64 the softmax denominator (bank per head)
                            nc.tensor.matmul(
                                pAV[0:65, 0:512],
                                v_sb[:, kt, p, 0, :], p_t[:, 0:512],
                                start=(kt == 0), stop=(kt == KT - 1))
                            nc.tensor.matmul(
                                pAV[0:65, 512:1024],
                                v_sb[:, kt, p, 1, :],
                                p_t[:, 512:1024],
                                start=(kt == 0), stop=(kt == KT - 1))
                            if kt == 4 and pending is not None:
                                emit_tail(*pending)
                                pending = None
                        rcp = emit_rcp(pAV)
                        pending = (p, qc, pAV, rcp)
                emit_tail(*pending)
                att_es.close()

                # ---- P4: out projection (accumulate over the 4 pairs)
                with tc.tile_pool(name="ps_o", bufs=3, space="PSUM") as ps_o, \
                     tc.tile_pool(name="wop", bufs=1) as wop, \
                     tc.tile_pool(name="osb", bufs=3) as osb:
                    wo_sb = wop.tile([128, NPAIR, D], BF16)
                    nc.sync.dma_start(wo_sb[:], wo_d[:])
                    for tt in range(NT):
                        if phases != "all":
                            # consume qT/kT so the projections aren't dead code
                            dm = osb.tile([128, 512], F32, tag="o")
                            nc.vector.tensor_tensor(
                                dm[:], qTf[:, 0, 0:512], kTf[:, 0, 0:512], OP.mult)
                            nc.sync.dma_start(
                                out_d[tt * 128:(tt + 1) * 128, 0:512], dm[:])
                            continue
                        for hf in range(2):
                            po = ps_o.tile([128, 512], F32, tag="o")
                            for p in range(NPAIR):
                                nc.tensor.matmul(
                                    po[:], attn_sb[:, p, tt * 128:(tt + 1) * 128],
                                    wo_sb[:, p, hf * 512:(hf + 1) * 512],
                                    start=(p == 0), stop=(p == NPAIR - 1))
                            o_sb = osb.tile([128, 512], F32, tag="o")
                            nc.vector.tensor_copy(o_sb[:], po[:])
                            nc.sync.dma_start(
                                out_d[tt * 128:(tt + 1) * 128, hf * 512:(hf + 1) * 512],
                                o_sb[:])
    return nc


def _get_program(loop_n=1, phases="all"):
    key = (loop_n, phases)
    if key not in _PROG:
        _install_bir_patch()
        _PROG[key] = _build_program(loop_n, phases)
    return _PROG[key]


def _make_in_maps(x, gamma, w_qkv, w_out):
    x = np.asarray(x, dtype=np.float32)
    gamma = np.asarray(gamma, dtype=np.float32)
    w_qkv = np.asarray(w_qkv, dtype=np.float32)
    w_out = np.asarray(w_out, dtype=np.float32)

    scale = gamma * np.float32(np.sqrt(D))          # fold sqrt(D)*gamma
    in_maps = []
    for core in range(8):
        b = core // 2
        hg = core % 2
        cols = slice(hg * 512, (hg + 1) * 512)
        wq = w_qkv[:, 0 * D:1 * D][:, cols] * scale[:, None] * np.float32(DH ** -0.5)
        wk = w_qkv[:, 1 * D:2 * D][:, cols] * scale[:, None]
        wv = w_qkv[:, 2 * D:3 * D][:, cols] * scale[:, None]
        wo = w_out[hg * 512:(hg + 1) * 512, :]
        import ml_dtypes
        bf16 = ml_dtypes.bfloat16
        in_maps.append({
            "x": np.ascontiguousarray(x[b]),
            "wq": np.ascontiguousarray(
                wq.reshape(FC, 128, 512).transpose(1, 0, 2).astype(bf16)),
            "wk": np.ascontiguousarray(
                wk.reshape(FC, 128, 512).transpose(1, 0, 2).astype(bf16)),
            "wv": np.ascontiguousarray(
                wv.reshape(FC, 128, 512).transpose(1, 0, 2).astype(bf16)),
            "wo": np.ascontiguousarray(
                wo.reshape(NPAIR, 128, D).transpose(1, 0, 2).astype(bf16)),
        })
    return in_maps


_RUNNER = None


def _build_runner(nc):
    """Persistent jitted callable over the 8-core mesh (avoids per-call
    re-tracing that run_bass_kernel_spmd incurs)."""
    import jax
    import concourse.mybir as mybir
    from jax.sharding import Mesh, PartitionSpec
    from jax.experimental.shard_map import shard_map
    from concourse.bass2jax import (
        _bass_exec_p, install_neuronx_cc_hook, partition_id_tensor)

    install_neuronx_cc_hook()
    partition_name = nc.partition_id_tensor.name if nc.partition_id_tensor else None
    in_names, out_names, out_avals, zero_shapes = [], [], [], []
    for alloc in nc.m.functions[0].allocations:
        if not isinstance(alloc, mybir.MemoryLocationSet):
            continue
        name = alloc.memorylocations[0].name
        if alloc.kind == "ExternalInput":
            if name != partition_name:
                in_names.append(name)
        elif alloc.kind == "ExternalOutput":
            out_names.append(name)
            shape = tuple(alloc.tensor_shape)
            dtype = mybir.dt.np(alloc.dtype)
            out_avals.append(jax.core.ShapedArray(shape, dtype))
            zero_shapes.append((shape, dtype))
    n_params = len(in_names)
    all_in_names = tuple(in_names + out_names)
    if partition_name is not None:
        all_in_names = all_in_names + (partition_name,)

    def _body(*args):
        operands = list(args)
        if partition_name is not None:
            operands.append(partition_id_tensor())
        return tuple(_bass_exec_p.bind(
            *operands,
            out_avals=tuple(out_avals),
            in_names=all_in_names,
            out_names=tuple(out_names),
            lowering_input_output_aliases=(),
            sim_require_finite=True,
            sim_require_nnan=True,
            nc=nc,
        ))

    devices = jax.devices()[:8]
    mesh = Mesh(np.asarray(devices), ("core",))
    nin = n_params + len(out_names)
    fn = jax.jit(shard_map(
        _body, mesh=mesh, in_specs=(PartitionSpec("core"),) * nin,
        out_specs=(PartitionSpec("core"),) * len(out_names), check_rep=False))

    def runner(in_maps):
        args = [np.concatenate([np.asarray(in_maps[c][nm]) for c in range(8)],
                               axis=0) for nm in in_names]
        for shape, dtype in zero_shapes:
            args.append(np.zeros((8 * shape[0], *shape[1:]), dtype))
        outs = fn(*args)
        o = np.asarray(outs[0]).reshape(8, T, D)
        return [o[c] for c in range(8)]

    return runner


def run(x, gamma, w_qkv, w_out, trace=False):
    """Run on the 8 NeuronCores; returns (output, results-or-None)."""
    global _RUNNER
    nc = _get_program()
    in_maps = _make_in_maps(x, gamma, w_qkv, w_out)
    res = None
    try:
        if _RUNNER is None:
            _RUNNER = _build_runner(nc)
        parts = _RUNNER(in_maps)
    except Exception:
        from concourse.bass_utils import run_bass_kernel_spmd
        res = run_bass_kernel_spmd(nc, in_maps, list(range(8)), trace=trace)
        parts = [res.results[i]["out"] for i in range(8)]
    out = np.stack([parts[2 * b] + parts[2 * b + 1] for b in range(B)], axis=0)
    return out, res


def kernel(x, gamma, w_qkv, w_out):
    out, _ = run(x, gamma, w_qkv, w_out, trace=False)
    return out

